# revision 28
# baseline (speedup 1.0000x reference)
"""Multi-head attention (B=2, N=2048, C=512, H=8) on 8 trn2 NeuronCores.

Sharding: tensor-parallel over heads x data-parallel over batch.
Core i handles batch b = i//4 and heads {2*(i%4), 2*(i%4)+1} (a contiguous
128-column slice of Wq/Wk/Wv and 128-row slice of Wo). Each core computes
its heads' full attention and a partial output projection; the host sums
the 4 partials per batch (adding bo once) and stacks batches.

v6 design (vs 200us fp32r v1 / 145us v3 / 139us v5):
  - Host pre-transposes q/kv/pos to channel-major and pre-casts all matmul
    operands to fp16 (no on-device input transposes, half the DMA bytes).
  - fp16 matmul operands everywhere, fp32 PSUM accumulation.
  - exp split across two engines per (kc, head) by parity: ACT table EXP
    vs a DVE Schraudolph exp (tensor_scalar -> int16 bit pattern IS fp16
    exp, 4% element error that softmax renormalization mostly cancels;
    measured end-to-end max-rel ~0.98e-2 vs the 2e-2 gate).
  - Phases 1+2 are fused: attention k-chunks are emitted as soon as their
    projection slabs exist (kc0-3 after slab1, kc4-7 after slab2, ...),
    so the PE never idles behind the input DMA stream.
  - One shared 4-bank PSUM work pool (2 lineages x bufs=2 of [128,512])
    carries projection groups, vp transposes, S tiles AND phase-3 Y
    tiles; + 2x[128,1024] PV accumulators = exactly 8 banks. S tiles are
    qq-granular so the exp chase never blocks the next S matmul.
  - PV lhsT padded to 128 cols ([d|ones|pad] stride-72) for fast weight
    load; ones column yields softmax denominators in the PV accumulator.
  - PV issue lags S/exp by PVLAG k-chunks; the normalize DVE work is
    deferred past the next q-half's first exps: no boundary stalls.
  - fp16 output partials, all output DMA on the sync queue.
"""
import numpy as np

B, N, C, H, D = 2, 2048, 512, 8, 64
SCALE = float(C) ** -0.5
NCORES = 8
P = 128
CC = C // P            # 4 channel chunks of 128
NT16 = N // P          # 16 token tiles of 128
NT4 = N // 512         # 4 token slabs of 512
PVLAG = 5              # PV issue lag in k-chunks

# Schraudolph fp16 exp constants (attention scale folded in):
#   i16 = rni(s * SCH_A + SCH_B); bitcast f16 ~= exp(s * SCALE)
SCH_A = SCALE * (2.0 ** 10) / float(np.log(2.0))
SCH_B = 15.0 * 1024.0 - 486411.0 / 8192.0

_cached_nc = None


def _build():
    from contextlib import ExitStack

    import concourse.mybir as mybir
    import concourse.tile as tile
    from concourse import bacc
    from concourse.alu_op_type import AluOpType
    from concourse.tile_rust import add_dep_helper

    f32 = mybir.dt.float32
    f16 = mybir.dt.float16
    i16 = mybir.dt.int16
    AF = mybir.ActivationFunctionType

    nc = bacc.Bacc("TRN2", target_bir_lowering=False, debug=False)

    qT = nc.dram_tensor("qT", [P, CC, N], f16, kind="ExternalInput")
    kvT = nc.dram_tensor("kvT", [P, CC, N], f16, kind="ExternalInput")
    posq = nc.dram_tensor("posq", [64, N], f16, kind="ExternalInput")
    posk = nc.dram_tensor("posk", [64, N], f16, kind="ExternalInput")
    wq = nc.dram_tensor("wq", [P, CC, P], f16, kind="ExternalInput")
    wk = nc.dram_tensor("wk", [P, CC, P], f16, kind="ExternalInput")
    wv = nc.dram_tensor("wv", [P, CC, P], f16, kind="ExternalInput")
    wo = nc.dram_tensor("wo", [P, C], f16, kind="ExternalInput")
    eye = nc.dram_tensor("eye", [P, P], f16, kind="ExternalInput")
    y = nc.dram_tensor("y", [N, C], f16, kind="ExternalOutput")

    with tile.TileContext(nc) as tc, ExitStack() as ctx:
        persist = ctx.enter_context(tc.tile_pool(name="persist", bufs=1))

        # ---- input DMAs: HWDGE queues carry only the critical stream ----
        qT_sb = persist.tile([P, CC, N], f16, tag="qT_sb")
        kvT_sb = persist.tile([P, CC, N], f16, tag="kvT_sb")
        w_sb = {name: persist.tile([P, CC, P], f16, tag=f"{name}_sb",
                                   name=f"{name}_sb")
                for name in ("wq", "wk", "wv")}
        pos_sb = {name: persist.tile([P, N], f16, tag=f"pos_{name}",
                                     name=f"pos_{name}")
                  for name in ("q", "k")}
        eye_sb = persist.tile([P, P], f16, tag="eye_sb")

        nc.sync.dma_start(w_sb["wq"][:], wq[:])
        nc.scalar.dma_start(w_sb["wk"][:], wk[:])
        nc.scalar.dma_start(w_sb["wv"][:], wv[:])
        for ntt in range(NT4):
            sl = slice(ntt * 512, (ntt + 1) * 512)
            nc.sync.dma_start(qT_sb[:, :, sl], qT[:, :, sl])
            nc.scalar.dma_start(kvT_sb[:, :, sl], kvT[:, :, sl])
            if ntt == 0:
                nc.sync.dma_start(pos_sb["q"][0:64, :], posq[:])
                nc.scalar.dma_start(pos_sb["k"][0:64, :], posk[:])
                # duplicate pos to partitions 64-127 (head-pair broadcast)
                nc.gpsimd.dma_start(pos_sb["q"][64:128, :], pos_sb["q"][0:64, :])
                nc.gpsimd.dma_start(pos_sb["k"][64:128, :], pos_sb["k"][0:64, :])
        nc.gpsimd.dma_start(eye_sb[:], eye[:])
        wo_r = []
        for h in (0, 1):
            wr = persist.tile([64, C], f16, tag=f"wo_r{h}")
            nc.gpsimd.dma_start(wr[:], wo[64 * h:64 * (h + 1), :])
            wo_r.append(wr)

        # PE order pinned with order-only deps (the PE queue is in-order).
        pe_prev = [None]

        def chain(mm):
            if pe_prev[0] is not None:
                add_dep_helper(mm.ins, pe_prev[0].ins, sync=False,
                               reason="pin PE order")
            pe_prev[0] = mm

        # ---- persistent activations ----
        qhT = persist.tile([P, N], f16, tag="qhT")
        khT = persist.tile([P, N], f16, tag="khT")
        O_sb = persist.tile([64, 2, N], f16, tag="O_sb")
        vpT = persist.tile([P, N], f16, tag="vpT")
        # vp layout per (kc, head): [d0..63 | ones] -> [128, 65] PV lhsT
        # (65-col weight loads: ~54ns, hidden under the prior MM's drain)
        vp4 = persist.tile([P, NT16, 2, 65], f16, tag="vp4")
        ones_col = persist.tile([P, 1], f16, tag="ones_col")
        nc.gpsimd.memset(ones_col[:], 1.0)
        nc.vector.tensor_copy(vp4[:, :, :, 64:65],
                              ones_col[:].to_broadcast((P, NT16, 2, 1)))

        # ---- pools: 4-bank shared work pool + 4-bank PV accumulators ----
        work_ps = ctx.enter_context(
            tc.tile_pool(name="work_ps", bufs=2, space="PSUM"))
        ot_ps = ctx.enter_context(
            tc.tile_pool(name="ot_ps", bufs=1, space="PSUM"))
        expp = ctx.enter_context(tc.tile_pool(name="expp", bufs=8))
        den_pool = ctx.enter_context(tc.tile_pool(name="den", bufs=2))
        den_dram = ctx.enter_context(
            tc.tile_pool(name="dend", bufs=2, space="DRAM"))
        yout = ctx.enter_context(tc.tile_pool(name="yout", bufs=4))

        wk_ctr = [0]

        def work_tile(shape, dt, name):
            tag = f"st{wk_ctr[0] % 2}"
            wk_ctr[0] += 1
            return work_ps.tile(shape, dt, tag=tag, name=name)

        # ---- phase-1 emission units (interleavable into the kc stream) --
        def emit_proj(ntt, wname):
            sl = slice(ntt * 512, (ntt + 1) * 512)
            srcT = qT_sb if wname == "wq" else kvT_sb
            pp = work_tile([P, 512], f32, f"pp_{wname}{ntt}")
            for cc in range(CC):
                chain(nc.tensor.matmul(
                    pp[:], w_sb[wname][:, cc], srcT[:, cc, sl],
                    start=(cc == 0), stop=(cc == CC - 1)))
            if wname == "wq":
                nc.vector.tensor_add(
                    out=qhT[:, sl], in0=pp[:], in1=pos_sb["q"][:, sl])
            elif wname == "wk":
                nc.vector.tensor_add(
                    out=khT[:, sl], in0=pp[:], in1=pos_sb["k"][:, sl])
            else:
                nc.scalar.copy(vpT[:, sl], pp[:])

        def emit_tp(t):
            tp = work_tile([P, P], f16, f"tp{t}")
            chain(nc.tensor.matmul(
                tp[:], vpT[:, t * P:(t + 1) * P], eye_sb[:],
                is_transpose=True))
            # on ACT: keeps the DVE queue clear for the Schraudolph exps
            nc.scalar.copy(
                vp4[:, t, :, 0:64],
                tp[:].rearrange("p (h d) -> p h d", h=2))

        def emit_slab(ntt):
            for wname in ("wq", "wk", "wv"):
                emit_proj(ntt, wname)
            for t in range(ntt * 4, ntt * 4 + 4):
                emit_tp(t)

        # ---- phase-2 emission ----
        st2 = {"ot": None, "exq": None}

        def s_quad(qh2, kc):
            # qq-outer order [h0q0, h1q0, h0q1, h1q1]: every LDWEIGHTS
            # overlaps the other head's in-flight matmul (disjoint PE row
            # groups), and head pairs can execute concurrently.
            exs = [expp.tile([P, 1024], f16, tag=f"ex{h}", name=f"ex{h}")
                   for h in (0, 1)]
            for qq in range(2):
                for h in (0, 1):
                    hsl = slice(64 * h, 64 * h + 64)
                    st = work_ps.tile([P, 512], f32, tag=f"st{h}",
                                      name=f"st{h}q{qq}")
                    chain(nc.tensor.matmul(
                        st[:],
                        khT[hsl, kc * P:(kc + 1) * P],
                        qhT[hsl, (qh2 * 2 + qq) * 512:
                                 (qh2 * 2 + qq + 1) * 512],
                        start=True, stop=True))
                    exq_sl = exs[h][:, qq * 512:(qq + 1) * 512]
                    if (kc + h) % 2 == 0:
                        nc.scalar.activation(exq_sl, st[:], AF.Exp,
                                             scale=SCALE)
                    else:
                        nc.vector.tensor_scalar(
                            exq_sl.bitcast(i16), st[:], SCH_A, SCH_B,
                            AluOpType.mult, AluOpType.add)
            return exs

        def pv_quad(kc, exs):
            OT = st2["ot"]
            for h in (0, 1):
                for qq in range(2):
                    chain(nc.tensor.matmul(
                        OT[h][:, qq * 512:(qq + 1) * 512],
                        vp4[:, kc, h, :],
                        exs[h][:, qq * 512:(qq + 1) * 512],
                        start=(kc == 0), stop=(kc == NT16 - 1)))

        def emit_kc(qh2, kc):
            if kc == 0:
                st2["ot"] = [ot_ps.tile([65, 1024], f32, tag=f"ot{h}",
                                        name=f"ot{h}") for h in (0, 1)]
                st2["exq"] = []
            st2["exq"].append(s_quad(qh2, kc))
            if kc >= PVLAG:
                pv_quad(kc - PVLAG, st2["exq"][kc - PVLAG])

        def drain_pv():
            for kc in range(NT16 - PVLAG, NT16):
                pv_quad(kc, st2["exq"][kc])

        # normalize part A: ACT den-row copies + DRAM-bounce broadcast
        def normalizeA(qh2):
            OT = st2["ot"]
            den_r = den_pool.tile([P, 2, 1024], f32, tag="den_r")
            for h in (0, 1):
                nc.scalar.copy(den_r[64:65, h, :], OT[h][64:65, :])
            den_d = den_dram.tile([1, 2, 1024], f32, tag="den_d")
            nc.sync.dma_start(den_d[:], den_r[64:65, :, :])
            den_bc = den_pool.tile([64, 2, 1024], f32, tag="den_bc")
            nc.sync.dma_start(den_bc[:], den_d[:].to_broadcast((64, 2, 1024)))
            return OT, den_bc

        # normalize part B: DVE reciprocal + per-head multiplies
        def normalizeB(qh2, OT, den_bc):
            q_sl = slice(qh2 * 1024, (qh2 + 1) * 1024)
            den_rec = den_pool.tile([64, 2, 1024], f32, tag="den_rec")
            nc.vector.reciprocal_approx_fast(den_rec[:], den_bc[:])
            for h in (0, 1):
                nc.vector.tensor_mul(
                    out=O_sb[:, h, q_sl], in0=OT[h][0:64, :],
                    in1=den_rec[:, h, :])

        # ---- fused emission schedule: phase-1 units injected between
        # early k-chunks so the PE fills the exp-chase windows ----
        emit_slab(0)
        emit_slab(1)
        inject = {0: [("proj", 2, "wq")], 1: [("proj", 2, "wk")],
                  2: [("proj", 2, "wv")], 3: [("tp", t) for t in (8, 9, 10, 11)],
                  4: [("proj", 3, "wq")], 5: [("proj", 3, "wk")],
                  6: [("proj", 3, "wv")], 7: [("tp", t) for t in (12, 13, 14, 15)]}
        for kc in range(NT16):
            emit_kc(0, kc)
            for unit in inject.get(kc, ()):
                if unit[0] == "proj":
                    emit_proj(unit[1], unit[2])
                else:
                    emit_tp(unit[1])
        drain_pv()
        norm0 = normalizeA(0)
        for kc in range(0, 3):
            emit_kc(1, kc)
        normalizeB(0, *norm0)
        for kc in range(3, NT16):
            emit_kc(1, kc)
        drain_pv()
        norm1 = normalizeA(1)
        normalizeB(1, *norm1)

        # ---- phase 3: output projection (partials, bias added on host) --
        # evacuate pairs of token tiles into one buffer -> 8 output DMAs
        for tt in range(NT16 // 2):
            ysb = yout.tile([P, 2, C], f16, tag="ysb")
            for ti in range(2):
                t = 2 * tt + ti
                tsl = slice(t * P, (t + 1) * P)
                yp = work_tile([P, C], f32, f"yp{t}")
                for h in (0, 1):
                    chain(nc.tensor.matmul(
                        yp[:], O_sb[:, h, tsl], wo_r[h][:],
                        start=(h == 0), stop=(h == 1)))
                if ti == 0:
                    nc.scalar.copy(ysb[:, 0, :], yp[:])
                else:
                    nc.vector.tensor_copy(ysb[:, 1, :], yp[:])
            nc.sync.dma_start(
                y[2 * tt * P:(2 * tt + 2) * P, :].rearrange(
                    "(a p) c -> p a c", a=2),
                ysb[:])

    nc.finalize()
    return nc


def _chmajor(x):
    # [N, C] token-major f32 -> [P, CC, N] channel-major chunked f16
    return np.ascontiguousarray(
        x.T.reshape(CC, P, N).transpose(1, 0, 2)).astype(np.float16)


def _wchunk(w):
    # [C, P] -> [P, CC, P] lhsT chunks
    return np.ascontiguousarray(
        w.reshape(CC, P, P).transpose(1, 0, 2)).astype(np.float16)


def _in_maps(q, kv, pos_q, pos_k, Wq, Wk, Wv, Wo, bo):
    maps = []
    for i in range(NCORES):
        b, hp = i // 4, i % 4
        cs = P * hp
        maps.append({
            "qT": _chmajor(np.asarray(q[b], dtype=np.float32)),
            "kvT": _chmajor(np.asarray(kv[b], dtype=np.float32)),
            "posq": np.ascontiguousarray(pos_q[b].T).astype(np.float16),
            "posk": np.ascontiguousarray(pos_k[b].T).astype(np.float16),
            "wq": _wchunk(np.asarray(Wq[:, cs:cs + P], dtype=np.float32)),
            "wk": _wchunk(np.asarray(Wk[:, cs:cs + P], dtype=np.float32)),
            "wv": _wchunk(np.asarray(Wv[:, cs:cs + P], dtype=np.float32)),
            "wo": np.ascontiguousarray(Wo[cs:cs + P, :]).astype(np.float16),
            "eye": np.eye(P, dtype=np.float16),
        })
    return maps


def kernel(q, kv, pos_q, pos_k, Wq, Wk, Wv, Wo, bo):
    from concourse.bass_utils import run_bass_kernel_spmd

    global _cached_nc
    if _cached_nc is None:
        _cached_nc = _build()

    args = [np.asarray(a) for a in (q, kv, pos_q, pos_k, Wq, Wk, Wv, Wo, bo)]
    maps = _in_maps(*args)
    res = run_bass_kernel_spmd(_cached_nc, maps, list(range(NCORES)))
    outs = [res.results[i]["y"].astype(np.float32) for i in range(NCORES)]
    bo32 = np.asarray(args[8], dtype=np.float32)
    y0 = outs[0] + outs[1] + outs[2] + outs[3] + bo32
    y1 = outs[4] + outs[5] + outs[6] + outs[7] + bo32
    return np.stack([y0, y1]).astype(np.float32)
